# revision 1
# baseline (speedup 1.0000x reference)
"""Trainium2 Bass kernel for nn_Graph_module_net_0_loss_type_18631568130084.

GNN message-passing block:
  gts       = relu(gt_feat @ Wg + bg)
  attn[i,j] = sigmoid(x[j]@Wq + x[i]@Wk + b_att)          (H == 1)
  atten     = (attn * (mr1+mr2) * col + f_diag) / CHILDS  ([B,H,Nj,Ni])
  o1 = relu(gconv1(x^T)); o1 += ln1(o1 @ atten)^T
  o2 = relu(gconv2(o1));  node_feat = ln2(o2 @ atten);  output2 = (o2 + node_feat^T)^T

Sharding: data-parallel over batch B=16 -> 2 batches per core on 8 cores.

Device-side layout notes:
 * Everything is built in the "transposed" (j-on-partitions) orientation so the
   PE contractions need no on-device big transposes except o1_new (32 small PE
   transposes).  Masks are pre-transposed/cast to fp16 on the host (pure layout
   transform; 0/1 values are exact in fp16) and m2^T is DMA-accumulated onto
   m1^T by the SWDGE engine, so atten^T costs one DVE pass + the sigmoids.
 * The global 1/CHILDS scale cancels inside both layernorms, so it is dropped
   and eps is rescaled by CHILDS^2 to keep the math exactly equivalent.
 * The top-k "col" mask is computed exactly on the host: a cheap sufficient
   condition (row-nonzeros <= k and every column touched by some mask nonzero)
   proves col == all-ones; otherwise an exact (slow) numpy replica runs.
"""

import numpy as np

B = 16
N = 1024
CIN = 256
MID = 512
OUT = 256
G = 4
CHILDS = 512
NCORES = 8
B_LOC = B // NCORES  # 2
NT = N // 128  # 8
EPS_LN = 1e-6 * float(CHILDS) ** 2  # eps rescaled because we drop the 1/CHILDS

F16 = np.float16
F32 = np.float32

_PROGRAM_CACHE = {}


def _build_program(beta1_nz: bool, beta2_nz: bool):
    import concourse.bacc as bacc
    import concourse.bass as bass
    import concourse.tile as tile
    from concourse import mybir

    f16 = mybir.dt.float16
    f32 = mybir.dt.float32
    AF = mybir.ActivationFunctionType
    OP = mybir.AluOpType

    nc = bacc.Bacc("TRN2", debug=False)

    def din(name, shape, dt):
        return nc.dram_tensor(name, shape, dt, kind="ExternalInput").ap()

    def dout(name, shape, dt):
        return nc.dram_tensor(name, shape, dt, kind="ExternalOutput").ap()

    # Per-core inputs (leading dim B_LOC where batch-dependent).
    m1T_d = din("m1T", [B_LOC, N, N], f16)       # masks_roi1^T  [j, i]
    m2T_d = din("m2T", [B_LOC, N, N], f16)       # masks_roi2^T  [j, i]
    xT_d = din("xT", [B_LOC, CIN, N], f16)       # x^T   [c, n]
    gtT_d = din("gtT", [B_LOC, CIN, N], f16)     # gt^T  [c, n]
    lirow_d = din("lirow", [B_LOC, N], f16)      # x@Wk + b_att      (per-i row)
    ljT_d = din("ljT", [B_LOC, 128, NT], f32)    # x@Wq chunked      (per-j bias)
    coljT_d = din("coljT", [B_LOC, 128, NT], f32)  # score*col chunked (per-j scale)
    fdiagT_d = din("fdiagT", [B_LOC, 128, NT], f32)  # (score==0) chunked
    # Replicated weights.
    wg_d = din("wgK", [2, 128, OUT], f16)        # Wg   (c-chunks)
    w1_d = din("w1K", [2, 128, MID], f16)        # block-diag W1^T (c-chunks)
    w2_d = din("w2K", [4, 128, OUT], f16)        # block-diag W2^T (m-chunks)
    bg_d = din("bgrow", [1, OUT], f16)
    b1_d = din("b1row", [1, MID], f16)
    b2_d = din("b2row", [1, OUT], f16)
    g1_d = din("g1row", [1, MID], f32)
    g2_d = din("g2row", [1, OUT], f32)
    beta1_d = din("beta1row", [1, MID], f32)
    beta2_d = din("beta2row", [1, OUT], f32)
    ident_d = din("ident", [128, 128], f16)
    ones_d = din("onescol", [1, 128], f16)

    gts_d = dout("gts", [B_LOC, N, OUT], f32)
    node_d = dout("node", [B_LOC, N, OUT], f32)
    out2_d = dout("out2", [B_LOC, N, OUT], f32)

    with tile.TileContext(nc) as tc:
        with tc.tile_pool(name="const", bufs=1) as constp, \
             tc.tile_pool(name="big", bufs=2) as bigp, \
             tc.tile_pool(name="work", bufs=4) as workp, \
             tc.tile_pool(name="outs", bufs=3) as outp, \
             tc.tile_pool(name="mm", bufs=4, space="PSUM") as mmp, \
             tc.tile_pool(name="tp", bufs=2, space="PSUM") as tpp:

            # ---- constants ----
            ident_t = constp.tile([128, 128], f16)
            nc.sync.dma_start(out=ident_t, in_=ident_d)
            ones_t = constp.tile([1, 128], f16)
            nc.sync.dma_start(out=ones_t, in_=ones_d)
            wg_t = constp.tile([128, 2, OUT], f16)
            nc.sync.dma_start(out=wg_t, in_=wg_d.rearrange("c p f -> p c f"))
            w1_t = constp.tile([128, 2, MID], f16)
            nc.sync.dma_start(out=w1_t, in_=w1_d.rearrange("c p f -> p c f"))
            w2_t = constp.tile([128, 4, OUT], f16)
            nc.sync.dma_start(out=w2_t, in_=w2_d.rearrange("c p f -> p c f"))
            bg_t = constp.tile([1, OUT], f16)
            nc.sync.dma_start(out=bg_t, in_=bg_d)
            b1_t = constp.tile([1, MID], f16)
            nc.sync.dma_start(out=b1_t, in_=b1_d)
            b2_t = constp.tile([1, OUT], f16)
            nc.sync.dma_start(out=b2_t, in_=b2_d)
            g1row_t = constp.tile([128, MID], f32)
            nc.sync.dma_start(out=g1row_t, in_=g1_d.to_broadcast([128, MID]))
            g2row_t = constp.tile([128, OUT], f32)
            nc.sync.dma_start(out=g2row_t, in_=g2_d.to_broadcast([128, OUT]))
            if beta1_nz:
                beta1_t = constp.tile([128, MID], f32)
                nc.sync.dma_start(out=beta1_t, in_=beta1_d.to_broadcast([128, MID]))
            if beta2_nz:
                beta2_t = constp.tile([128, OUT], f32)
                nc.sync.dma_start(out=beta2_t, in_=beta2_d.to_broadcast([128, OUT]))
            eps_t = constp.tile([128, 1], f32)
            nc.vector.memset(eps_t, EPS_LN)

            for b in range(B_LOC):
                # ---- per-batch small tiles ----
                lirow_t = workp.tile([128, N], f16, tag="lirow")
                nc.sync.dma_start(
                    out=lirow_t, in_=lirow_d[b : b + 1, :].to_broadcast([128, N])
                )
                ljT_t = workp.tile([128, NT], f32, tag="ljT")
                nc.sync.dma_start(out=ljT_t, in_=ljT_d[b])
                coljT_t = workp.tile([128, NT], f32, tag="coljT")
                nc.sync.dma_start(out=coljT_t, in_=coljT_d[b])
                fdiagT_t = workp.tile([128, NT], f32, tag="fdiagT")
                nc.sync.dma_start(out=fdiagT_t, in_=fdiagT_d[b])
                xT_t = bigp.tile([128, 2, N], f16, tag="xT")
                nc.sync.dma_start(
                    out=xT_t, in_=xT_d[b].rearrange("(c p) n -> p c n", p=128)
                )
                gtT_t = bigp.tile([128, 2, N], f16, tag="gtT")
                nc.sync.dma_start(
                    out=gtT_t, in_=gtT_d[b].rearrange("(c p) n -> p c n", p=128)
                )

                At = bigp.tile([128, NT, N], f16, tag="At")      # atten^T [j, i]
                o1t = bigp.tile([128, NT, MID], f16, tag="o1t")  # o1^T    [j, m]
                o1nT = bigp.tile([128, NT, MID], f16, tag="o1nT")  # o1_new^T [n, m]
                o1n = bigp.tile([128, 4, N], f16, tag="o1n")     # o1_new  [m, j]
                o2t = bigp.tile([128, NT, OUT], f16, tag="o2t")  # o2^T    [j, o]

                # ---- stage A: atten^T ----
                for jt in range(NT):
                    ms = workp.tile([128, N], f16, tag="ms")
                    nc.sync.dma_start(out=ms, in_=m1T_d[b, jt * 128 : (jt + 1) * 128, :])
                    nc.gpsimd.dma_start(
                        out=ms,
                        in_=m2T_d[b, jt * 128 : (jt + 1) * 128, :],
                        accum_op=OP.add,
                    )
                    sg = workp.tile([128, N], f16, tag="sg")
                    nc.scalar.activation(
                        out=sg, in_=lirow_t, func=AF.Sigmoid,
                        bias=ljT_t[:, jt : jt + 1], scale=1.0,
                    )
                    # atten^T = (m1T+m2T) * (score*col per-j) * sigmoid^T
                    nc.vector.scalar_tensor_tensor(
                        out=At[:, jt, :], in0=ms, scalar=coljT_t[:, jt : jt + 1],
                        in1=sg, op0=OP.mult, op1=OP.mult,
                    )
                    # diagonal += (score==0)
                    dtile = workp.tile([128, 128], f16, tag="dtile")
                    nc.vector.tensor_scalar_mul(dtile, ident_t, fdiagT_t[:, jt : jt + 1])
                    nc.vector.tensor_add(
                        At[:, jt, jt * 128 : (jt + 1) * 128],
                        At[:, jt, jt * 128 : (jt + 1) * 128],
                        dtile,
                    )

                # ---- stage B: gts ----
                for nt in range(NT):
                    ps = mmp.tile([128, MID], mybir.dt.float32, tag="ps")
                    p256 = ps[:, :OUT]
                    nc.tensor.matmul(p256, lhsT=ones_t, rhs=bg_t, start=True, stop=False)
                    for cc in range(2):
                        nc.tensor.matmul(
                            p256,
                            lhsT=gtT_t[:, cc, nt * 128 : (nt + 1) * 128],
                            rhs=wg_t[:, cc, :],
                            start=False, stop=(cc == 1),
                        )
                    gto = outp.tile([128, OUT], f32, tag="gto")
                    nc.scalar.activation(out=gto, in_=p256, func=AF.Relu)
                    nc.scalar.dma_start(
                        out=gts_d[b, nt * 128 : (nt + 1) * 128, :], in_=gto
                    )

                # ---- stage C: gconv1 -> o1^T ----
                for jt in range(NT):
                    ps = mmp.tile([128, MID], mybir.dt.float32, tag="ps")
                    nc.tensor.matmul(ps, lhsT=ones_t, rhs=b1_t, start=True, stop=False)
                    for cc in range(2):
                        nc.tensor.matmul(
                            ps,
                            lhsT=xT_t[:, cc, jt * 128 : (jt + 1) * 128],
                            rhs=w1_t[:, cc, :],
                            start=False, stop=(cc == 1),
                        )
                    nc.scalar.activation(out=o1t[:, jt, :], in_=ps, func=AF.Relu)

                # ---- stage D: o1m^T = atten^T-contraction, ln1, residual ----
                for it in range(NT):
                    ps = mmp.tile([128, MID], mybir.dt.float32, tag="ps")
                    for jc in range(NT):
                        nc.tensor.matmul(
                            ps,
                            lhsT=At[:, jc, it * 128 : (it + 1) * 128],
                            rhs=o1t[:, jc, :],
                            start=(jc == 0), stop=(jc == NT - 1),
                        )
                    sv = workp.tile([128, 6], f32, tag="sv")
                    nc.vector.bn_stats(out=sv, in_=ps)
                    mv = workp.tile([128, 2], f32, tag="mv")
                    nc.vector.bn_aggr(out=mv, in_=sv)
                    std = workp.tile([128, 1], f32, tag="std")
                    nc.scalar.activation(
                        out=std, in_=mv[:, 1:2], func=AF.Sqrt, bias=eps_t
                    )
                    rstd = workp.tile([128, 1], f32, tag="rstd")
                    nc.vector.reciprocal(out=rstd, in_=std)
                    outer = workp.tile([128, MID], f16, tag="outer")
                    nc.vector.tensor_scalar_mul(outer, g1row_t, rstd)
                    ln = workp.tile([128, MID], f16, tag="ln")
                    nc.vector.scalar_tensor_tensor(
                        out=ln, in0=ps, scalar=mv[:, 0:1], in1=outer,
                        op0=OP.subtract, op1=OP.mult,
                    )
                    if beta1_nz:
                        nc.vector.tensor_add(ln, ln, beta1_t)
                    nc.vector.tensor_add(o1nT[:, it, :], ln, o1t[:, it, :])

                # ---- stage E: transpose o1_new, gconv2 -> o2^T ----
                for mc in range(4):
                    tp = tpp.tile([128, N], f16, tag="tp")
                    for it in range(NT):
                        nc.tensor.transpose(
                            tp[:, it * 128 : (it + 1) * 128],
                            o1nT[:, it, mc * 128 : (mc + 1) * 128],
                            ident_t,
                        )
                    for h in range(2):
                        nc.scalar.activation(
                            out=o1n[:, mc, h * 512 : (h + 1) * 512],
                            in_=tp[:, h * 512 : (h + 1) * 512],
                            func=AF.Copy,
                        )
                for jt in range(NT):
                    ps = mmp.tile([128, MID], mybir.dt.float32, tag="ps")
                    p256 = ps[:, :OUT]
                    nc.tensor.matmul(p256, lhsT=ones_t, rhs=b2_t, start=True, stop=False)
                    for mc in range(4):
                        nc.tensor.matmul(
                            p256,
                            lhsT=o1n[:, mc, jt * 128 : (jt + 1) * 128],
                            rhs=w2_t[:, mc, :],
                            start=False, stop=(mc == 3),
                        )
                    nc.scalar.activation(out=o2t[:, jt, :], in_=p256, func=AF.Relu)

                # ---- stage F: o2m^T, ln2 -> node_feat, output2 ----
                for it in range(NT):
                    ps = mmp.tile([128, MID], mybir.dt.float32, tag="ps")
                    p256 = ps[:, :OUT]
                    for jc in range(NT):
                        nc.tensor.matmul(
                            p256,
                            lhsT=At[:, jc, it * 128 : (it + 1) * 128],
                            rhs=o2t[:, jc, :],
                            start=(jc == 0), stop=(jc == NT - 1),
                        )
                    sv = workp.tile([128, 6], f32, tag="sv")
                    nc.vector.bn_stats(out=sv, in_=p256)
                    mv = workp.tile([128, 2], f32, tag="mv")
                    nc.vector.bn_aggr(out=mv, in_=sv)
                    std = workp.tile([128, 1], f32, tag="std")
                    nc.scalar.activation(
                        out=std, in_=mv[:, 1:2], func=AF.Sqrt, bias=eps_t
                    )
                    rstd = workp.tile([128, 1], f32, tag="rstd")
                    nc.vector.reciprocal(out=rstd, in_=std)
                    outer2 = workp.tile([128, OUT], f16, tag="outer2")
                    nc.vector.tensor_scalar_mul(outer2, g2row_t, rstd)
                    nf = outp.tile([128, OUT], f32, tag="nf")
                    nc.vector.scalar_tensor_tensor(
                        out=nf, in0=p256, scalar=mv[:, 0:1], in1=outer2,
                        op0=OP.subtract, op1=OP.mult,
                    )
                    if beta2_nz:
                        nc.vector.tensor_add(nf, nf, beta2_t)
                    nc.scalar.dma_start(
                        out=node_d[b, it * 128 : (it + 1) * 128, :], in_=nf
                    )
                    o2o = outp.tile([128, OUT], f32, tag="o2o")
                    nc.vector.tensor_add(o2o, nf, o2t[:, it, :])
                    nc.scalar.dma_start(
                        out=out2_d[b, it * 128 : (it + 1) * 128, :], in_=o2o
                    )

    nc.compile()
    return nc


def _compute_col_fast(m1, m2, sm):
    """Exact col == ones proof via a cheap sufficient condition, else None."""
    if m1.min() < 0.0 or m2.min() < 0.0 or sm.min() < 0.0:
        return None
    spos = (sm > 0).astype(F32)
    colnz = np.zeros(N, dtype=bool)
    nz1max = 0.0
    nz2max = 0.0
    for b in range(B):
        p1 = (m1[b] > 0).astype(F32)
        p2 = (m2[b] > 0).astype(F32)
        nz1max = max(nz1max, float((p1 @ spos[b]).max()))
        nz2max = max(nz2max, float((p2 @ spos[b]).max()))
        colnz |= ((p1 + p2).max(axis=0) > 0) & (spos[b] > 0)
    if nz1max <= CHILDS // 4 and nz2max <= CHILDS // 2 and colnz.all():
        return np.ones(N, dtype=F32)
    return None


def _compute_col_slow(m1, m2, sm, li, lj):
    """Exact replica of the reference top-k column-union (numpy)."""
    k4, k2 = CHILDS // 4, CHILDS // 2
    col = np.zeros(N, dtype=bool)
    for b in range(B):
        logits = li[b][:, None] + lj[b][None, :]
        a = 1.0 / (1.0 + np.exp(-logits.astype(F32)))
        mr1 = m1[b] * sm[b][None, :]
        mr2 = m2[b] * sm[b][None, :]
        a1 = a * mr1
        a2 = a * mr2
        # lax.top_k ties -> lowest index; stable argsort on (-a) reproduces it.
        col[np.argsort(-a1, axis=1, kind="stable")[:, :k4].ravel()] = True
        col[np.argsort(a1, axis=1, kind="stable")[:, :k4].ravel()] = True
        col[np.argsort(-a2, axis=1, kind="stable")[:, :k2].ravel()] = True
        col[np.argsort(a2, axis=1, kind="stable")[:, :k4].ravel()] = True
    return col.astype(F32)


def kernel(**inputs):
    x = np.ascontiguousarray(np.asarray(inputs["x"], dtype=F32))
    m1 = np.asarray(inputs["masks_roi1"], dtype=F32)
    m2 = np.asarray(inputs["masks_roi2"], dtype=F32)
    sm = np.asarray(inputs["score_mask"], dtype=F32)
    gt = np.asarray(inputs["gt_feat"], dtype=F32)
    W_att = np.asarray(inputs["W_att"], dtype=F32)
    b_att = np.asarray(inputs["b_att"], dtype=F32)
    W1 = np.asarray(inputs["W1"], dtype=F32)
    b1 = np.asarray(inputs["b1"], dtype=F32)
    W2 = np.asarray(inputs["W2"], dtype=F32)
    b2 = np.asarray(inputs["b2"], dtype=F32)
    g1 = np.asarray(inputs["g1"], dtype=F32)
    beta1 = np.asarray(inputs["beta1"], dtype=F32)
    g2 = np.asarray(inputs["g2"], dtype=F32)
    beta2 = np.asarray(inputs["beta2"], dtype=F32)
    Wg = np.asarray(inputs["Wg"], dtype=F32)
    bg = np.asarray(inputs["bg"], dtype=F32)

    assert x.shape == (B, N, CIN) and W_att.shape == (2 * CIN, 1)

    # ---- host prep: tiny vector math + layout/dtype staging ----
    lj = x.reshape(B * N, CIN) @ W_att[:CIN, 0]
    lj = lj.reshape(B, N)
    li = x.reshape(B * N, CIN) @ W_att[CIN:, 0]
    li = li.reshape(B, N) + b_att[0]

    col = _compute_col_fast(m1, m2, sm)
    if col is None:
        col = _compute_col_slow(m1, m2, sm, li, lj)

    m1T = np.ascontiguousarray(m1.transpose(0, 2, 1)).astype(F16)
    m2T = np.ascontiguousarray(m2.transpose(0, 2, 1)).astype(F16)
    xT = np.ascontiguousarray(x.transpose(0, 2, 1)).astype(F16)
    gtT = np.ascontiguousarray(gt.transpose(0, 2, 1)).astype(F16)
    lirow = li.astype(F16)
    ljT = np.ascontiguousarray(lj.reshape(B, NT, 128).transpose(0, 2, 1)).astype(F32)
    colj = sm * col[None, :]
    coljT = np.ascontiguousarray(colj.reshape(B, NT, 128).transpose(0, 2, 1)).astype(F32)
    fd = (sm == 0).astype(F32)
    fdiagT = np.ascontiguousarray(fd.reshape(B, NT, 128).transpose(0, 2, 1)).astype(F32)

    # Weights: block-diagonal transposed layouts for the grouped convs.
    w1bd = np.zeros((CIN, MID), dtype=F32)
    for g in range(G):
        w1bd[64 * g : 64 * (g + 1), 128 * g : 128 * (g + 1)] = W1[
            128 * g : 128 * (g + 1), :
        ].T
    w2bd = np.zeros((MID, OUT), dtype=F32)
    for g in range(G):
        w2bd[128 * g : 128 * (g + 1), 64 * g : 64 * (g + 1)] = W2[
            64 * g : 64 * (g + 1), :
        ].T
    wgK = np.ascontiguousarray(Wg.reshape(2, 128, OUT)).astype(F16)
    w1K = np.ascontiguousarray(w1bd.reshape(2, 128, MID)).astype(F16)
    w2K = np.ascontiguousarray(w2bd.reshape(4, 128, OUT)).astype(F16)

    shared = {
        "wgK": wgK,
        "w1K": w1K,
        "w2K": w2K,
        "bgrow": bg.reshape(1, OUT).astype(F16),
        "b1row": b1.reshape(1, MID).astype(F16),
        "b2row": b2.reshape(1, OUT).astype(F16),
        "g1row": g1.reshape(1, MID).astype(F32),
        "g2row": g2.reshape(1, OUT).astype(F32),
        "beta1row": beta1.reshape(1, MID).astype(F32),
        "beta2row": beta2.reshape(1, OUT).astype(F32),
        "ident": np.eye(128, dtype=F16),
        "onescol": np.ones((1, 128), dtype=F16),
    }
    in_maps = []
    for c in range(NCORES):
        s = slice(B_LOC * c, B_LOC * (c + 1))
        in_maps.append(
            {
                "m1T": m1T[s],
                "m2T": m2T[s],
                "xT": xT[s],
                "gtT": gtT[s],
                "lirow": lirow[s],
                "ljT": ljT[s],
                "coljT": coljT[s],
                "fdiagT": fdiagT[s],
                **shared,
            }
        )

    beta_key = (bool(np.any(beta1)), bool(np.any(beta2)))
    if beta_key not in _PROGRAM_CACHE:
        _PROGRAM_CACHE[beta_key] = _build_program(*beta_key)
    nc = _PROGRAM_CACHE[beta_key]

    global _LAST_IN_MAPS
    _LAST_IN_MAPS = in_maps

    from concourse.bass_utils import run_bass_kernel_spmd

    res = run_bass_kernel_spmd(nc, in_maps, core_ids=list(range(NCORES)))
    results = res.results if hasattr(res, "results") else res

    output2 = np.concatenate([results[c]["out2"] for c in range(NCORES)], axis=0)
    gts = np.concatenate([results[c]["gts"] for c in range(NCORES)], axis=0)
    node_feat = np.concatenate([results[c]["node"] for c in range(NCORES)], axis=0)
    return output2.astype(F32), gts.astype(F32), node_feat.astype(F32)

